# revision 18
# baseline (speedup 1.0000x reference)
"""Trainium2 Bass kernel for batched multi-mask masked-mean (segment_reduce).

Computes, for each (batch, area) pair and each of two mask tensors:
    m   = smooth-AND over 4 channels of differentiable_eq(mask, initial_mask_id)
    out = m * (sum(m * img) / sum(m))        (masked mean over the 16x16 patch)

Sharding: data-parallel over the flattened (batch * n_areas) axis across 8
NeuronCores; no cross-core communication.

Math notes:
  diff_round(x) = x - sin(2*pi*x)/(2*pi).  Work in "y-space" (y = 2*pi*x):
  f(y) = y - sin(y); harder_diff_round(x) = f(f(f(2*pi*x)))/(2*pi).
  The ScalarEngine Sin spline is valid only on [-pi, pi], so every sin(y) for
  y in [0, 2*pi] is computed as -sin(y - pi) via the activation's free affine
  (bias = -pi), turning all f-step subtracts into adds.
  differentiable_eq(a, B) with B = hdr(id) constant per (area, channel) is the
  affine  t = A*(2B-1) + (1-B)  of A = hdr(a); in y-space z = yA*S + U with
  S = 2B-1, U = 2*pi*(1-B), both precomputed on host (tiny).
  The masked mean is scale-invariant in m, so the pipeline carries
  m~ = (2*pi)^2 * m and only rescales in the final per-area multiply.
"""

import itertools

import numpy as np

import concourse.bacc as bacc
import concourse.mybir as mybir
import concourse.tile as tile
from concourse.bass_utils import run_bass_kernel_spmd

# ---------------------------------------------------------------- geometry
N_CORES = 8
B, N, DX, DY, C = 2, 8192, 16, 16, 4
PIX = DX * DY                      # 256 pixels per area
W_IN = PIX * C                     # 1024 mask values per area (channel-interleaved)
A_TOT = B * N                      # 16384 areas
A_CORE = A_TOT // N_CORES          # 2048 areas per core
P = 128                            # SBUF partitions

PI = float(np.pi)
TWO_PI = float(2.0 * np.pi)
EPS_GUARD = 2e-5                   # keeps sin args strictly inside [-pi, pi]
GA = 1.0 - EPS_GUARD
INV_4PI2 = float(1.0 / (4.0 * np.pi * np.pi))

F32 = mybir.dt.float32
BF16 = mybir.dt.bfloat16
SIN = mybir.ActivationFunctionType.Sin
COPY = mybir.ActivationFunctionType.Copy
MULT = mybir.AluOpType.mult
ADD = mybir.AluOpType.add
BYPASS = mybir.AluOpType.bypass
AX_X = mybir.AxisListType.X

# compute dtype for the bulk elementwise pipeline ("f32" or "bf16")
COMPUTE = "f32"
G = 2                              # areas per partition per mega-tile
BIG_BUFS = 4
MED_BUFS = 3
AND_BF16 = True                    # AND phase (w products onward) in bf16
CCE_STEPS = ("e3",)                # f-step adds computed by DMA CCE accumulate


def build(nc, a_core=A_CORE, g=G, compute=COMPUTE):
    """Emit the Tile graph onto `nc` for one core's shard of `a_core` areas."""
    dt = F32 if compute == "f32" else BF16
    W = g * W_IN                   # mega-tile mask width (f32 elems per partition)
    Q = g * PIX                    # mega-tile single-channel width
    n_tiles = a_core // (P * g)
    assert n_tiles * P * g == a_core

    d_mask = nc.dram_tensor("mask", [a_core, W_IN], F32, kind="ExternalInput")
    d_alt = nc.dram_tensor("alt", [a_core, W_IN], F32, kind="ExternalInput")
    d_img = nc.dram_tensor("img", [a_core, PIX], F32, kind="ExternalInput")
    d_su = nc.dram_tensor("su", [a_core, 8], F32, kind="ExternalInput")
    d_out = nc.dram_tensor("out", [a_core, PIX], F32, kind="ExternalOutput")
    d_outa = nc.dram_tensor("outalt", [a_core, PIX], F32, kind="ExternalOutput")

    mask_v = d_mask.ap().rearrange("(t p g) f -> t p (g f)", p=P, g=g)
    alt_v = d_alt.ap().rearrange("(t p g) f -> t p (g f)", p=P, g=g)
    img_v = d_img.ap().rearrange("(t p g) f -> t p (g f)", p=P, g=g)
    su_v = d_su.ap().rearrange("(t p g) c -> p t g c", p=P, g=g)
    out_v = d_out.ap().rearrange("(t p g) f -> t p (g f)", p=P, g=g)
    outa_v = d_outa.ap().rearrange("(t p g) f -> t p (g f)", p=P, g=g)

    with tile.TileContext(nc) as tc:
        from contextlib import ExitStack

        with ExitStack() as ctx:
            const = ctx.enter_context(tc.tile_pool(name="const", bufs=1))
            big = ctx.enter_context(tc.tile_pool(name="big", bufs=BIG_BUFS))
            med = ctx.enter_context(tc.tile_pool(name="med", bufs=MED_BUFS))
            sm = ctx.enter_context(tc.tile_pool(name="sm", bufs=MED_BUFS))

            nb = const.tile([P, 1], F32, tag="nb")       # -pi*GA bias for sin
            nc.gpsimd.memset(nb[:], -PI * GA)
            su_sb = const.tile([P, n_tiles * g * 8], F32, tag="su")
            nc.sync.dma_start(
                su_sb[:].rearrange("p (t g c) -> p t g c", t=n_tiles, g=g), su_v
            )

            def f_step(y, tag, out_dt=None, cce=False):
                """y <- f(y) = y - sin(y), via s = -sin(y) then add."""
                s = big.tile([P, W], out_dt or dt, tag="sin")
                nc.scalar.activation(s[:], y[:], SIN, scale=GA, bias=nb[:])
                if cce:
                    # accumulate in place on the DMA engines (CCE inline add);
                    # frees the VectorEngine at the cost of SBUF fabric traffic
                    nc.gpsimd.dma_start(y[:], s[:], accum_op=ADD)
                    return y
                y2 = big.tile([P, W], out_dt or dt, tag=tag)
                nc.vector.tensor_tensor(y2[:], y[:], s[:], ADD)
                return y2

            def emit_pass(t, j, img_c):
                src_v, dst_v = ((mask_v, out_v), (alt_v, outa_v))[j]
                x = big.tile([P, W], F32, tag="x")
                nc.sync.dma_start(x[:], src_v[t])

                # ---- A phase: y3 = f^3(2*pi*x)  (hdr of mask, y-space).
                # The y1 stt writes channel-major [c][g][pix] (free output
                # permutation), so the eq-phase tensor_scalars read/write
                # contiguous slices and hit the 2x perf mode.
                s0 = big.tile([P, W], dt, tag="sin")
                nc.scalar.activation(s0[:], x[:], SIN, scale=TWO_PI * GA, bias=nb[:])
                y1 = big.tile([P, W], dt, tag="yy")
                y1_cm = y1[:].rearrange("p (c g i) -> p g i c", c=C, g=g)
                x_il = x[:].rearrange("p (g i c) -> p g i c", g=g, c=C)
                s0_il = s0[:].rearrange("p (g i c) -> p g i c", g=g, c=C)
                if compute == "f32":
                    nc.vector.scalar_tensor_tensor(
                        y1_cm, x_il, TWO_PI, s0_il, MULT, ADD
                    )
                else:
                    y0 = big.tile([P, W], dt, tag="y0")
                    nc.scalar.activation(y0[:], x[:], COPY, scale=TWO_PI)
                    nc.vector.tensor_tensor(
                        y1_cm, y0[:].rearrange("p (g i c) -> p g i c", g=g, c=C),
                        s0_il, ADD,
                    )
                y2 = f_step(y1, "yy", cce="y2" in CCE_STEPS)
                y3 = f_step(y2, "yy")
                yield

                # ---- eq phase: z = y3*S + U per (area, channel); y3 is
                # already channel-major so every slice is contiguous
                z = big.tile([P, W], dt, tag="zz")
                for gg in range(g):
                    col = (t * g + gg) * 8
                    for c in range(C):
                        cs = slice((c * g + gg) * PIX, (c * g + gg + 1) * PIX)
                        nc.vector.tensor_scalar(
                            z[:, cs],
                            y3[:, cs],
                            su_sb[:, col + c : col + c + 1],
                            su_sb[:, col + 4 + c : col + 4 + c + 1],
                            MULT,
                            ADD,
                        )
                # f^3 -> e (y-space eq), then w = f(e) = 2*pi*dr(eq)
                e1 = f_step(z, "zz")
                e2 = f_step(e1, "zz")
                e3 = f_step(e2, "zz", cce="e3" in CCE_STEPS)
                w = f_step(e3, "zz", out_dt=BF16 if AND_BF16 else None)
                yield

                # ---- AND phase (channel-major blocks are contiguous)
                adt = BF16 if AND_BF16 else dt
                ab = med.tile([P, 2 * Q], adt, tag="ab")
                nc.vector.tensor_tensor(ab[:, 0:Q], w[:, 0:Q], w[:, Q : 2 * Q], MULT)
                nc.vector.tensor_tensor(
                    ab[:, Q : 2 * Q], w[:, 2 * Q : 3 * Q], w[:, 3 * Q : 4 * Q], MULT
                )
                sab = med.tile([P, 2 * Q], adt, tag="sab")
                nc.scalar.activation(sab[:], ab[:], SIN, scale=GA / TWO_PI, bias=nb[:])
                fab = med.tile([P, 2 * Q], adt, tag="fab")
                nc.vector.scalar_tensor_tensor(
                    fab[:], ab[:], 1.0 / TWO_PI, sab[:], MULT, ADD
                )
                fa, fb = fab[:, 0:Q], fab[:, Q : 2 * Q]

                den = sm.tile([P, g], F32, tag="den")
                num = sm.tile([P, g], F32, tag="num")
                m = med.tile([P, Q], adt, tag="mm")
                mi = med.tile([P, Q], adt, tag="mi")
                for gg in range(g):
                    gs = slice(gg * PIX, (gg + 1) * PIX)
                    nc.vector.scalar_tensor_tensor(
                        m[:, gs], fa[:, gs], 0.0, fb[:, gs], BYPASS, MULT,
                        accum_out=den[:, gg : gg + 1],
                    )
                    nc.vector.scalar_tensor_tensor(
                        mi[:, gs], m[:, gs], 0.0, img_c[:, gs], BYPASS, MULT,
                        accum_out=num[:, gg : gg + 1],
                    )
                rd = sm.tile([P, g], F32, tag="rd")
                nc.vector.reciprocal(rd[:], den[:])
                q = sm.tile([P, g], F32, tag="qq")
                nc.vector.tensor_tensor(q[:], num[:], rd[:], MULT)

                o = med.tile([P, Q], F32, tag="oo")
                for gg in range(g):
                    nc.vector.tensor_scalar(
                        o[:, gg * PIX : (gg + 1) * PIX],
                        m[:, gg * PIX : (gg + 1) * PIX],
                        q[:, gg : gg + 1],
                        INV_4PI2,
                        MULT,
                        MULT,
                    )
                nc.sync.dma_start(dst_v[t], o[:])
                yield

            for t in range(n_tiles):
                img_sb = med.tile([P, Q], F32, tag="img")
                nc.sync.dma_start(img_sb[:], img_v[t])
                if AND_BF16 or compute != "f32":
                    img_c = med.tile([P, Q], BF16 if AND_BF16 else dt, tag="imgc")
                    nc.vector.tensor_copy(img_c[:], img_sb[:])
                else:
                    img_c = img_sb
                # interleave the two independent mask pipelines phase-by-phase
                for _ in itertools.zip_longest(
                    emit_pass(t, 0, img_c), emit_pass(t, 1, img_c)
                ):
                    pass

    return nc


# ------------------------------------------------------------- host helpers
def _hdr_np(x):
    def dr(v):
        return v - np.sin(2.0 * np.pi * v) / (2.0 * np.pi)

    return dr(dr(dr(x)))


_NC_CACHE = {}


def _get_compiled():
    key = (COMPUTE, G)
    if key not in _NC_CACHE:
        nc = bacc.Bacc(
            "TRN2", target_bir_lowering=False, debug=False, num_devices=N_CORES
        )
        build(nc, A_CORE, G, COMPUTE)
        nc.compile()
        _NC_CACHE[key] = nc
    return _NC_CACHE[key]


def _make_in_maps(resized_image, mask_combined, mask_combined_alt, initial_mask_id):
    mask = np.ascontiguousarray(
        np.asarray(mask_combined, dtype=np.float32).reshape(A_TOT, W_IN)
    )
    alt = np.ascontiguousarray(
        np.asarray(mask_combined_alt, dtype=np.float32).reshape(A_TOT, W_IN)
    )
    img = np.ascontiguousarray(
        np.asarray(resized_image, dtype=np.float32).reshape(A_TOT, PIX)
    )
    idf = np.asarray(initial_mask_id, dtype=np.float64).reshape(A_TOT, C)
    bh = _hdr_np(idf)
    su = np.concatenate(
        [2.0 * bh - 1.0, 2.0 * np.pi * (1.0 - bh)], axis=1
    ).astype(np.float32)

    in_maps = []
    for k in range(N_CORES):
        sl = slice(k * A_CORE, (k + 1) * A_CORE)
        in_maps.append(
            {"mask": mask[sl], "alt": alt[sl], "img": img[sl], "su": su[sl]}
        )
    return in_maps


def run(inputs, trace=False, trace_kwargs=None):
    """Run the kernel on all 8 cores; returns ((out, out_alt), exec_time_ns)."""
    nc = _get_compiled()
    in_maps = _make_in_maps(
        inputs["resized_image"],
        inputs["mask_combined"],
        inputs["mask_combined_alt"],
        inputs["initial_mask_id"],
    )
    res = run_bass_kernel_spmd(
        nc,
        in_maps,
        list(range(N_CORES)),
        trace=trace,
        **(trace_kwargs or {}),
    )
    out = np.empty((A_TOT, PIX), np.float32)
    outa = np.empty((A_TOT, PIX), np.float32)
    for k in range(N_CORES):
        sl = slice(k * A_CORE, (k + 1) * A_CORE)
        out[sl] = res.results[k]["out"]
        outa[sl] = res.results[k]["outalt"]
    shape = (B, N, DX, DY, 1)
    return (out.reshape(shape), outa.reshape(shape)), res.exec_time_ns


def kernel(**inputs):
    (out, outa), _ = run(inputs, trace=False)
    return out, outa


# revision 22
# speedup vs baseline: 1.8181x; 1.8181x over previous
"""Trainium2 Bass kernel for batched multi-mask masked-mean (segment_reduce).

Computes, for each (batch, area) pair and each of two mask tensors:
    m   = smooth-AND over 4 channels of differentiable_eq(mask, initial_mask_id)
    out = m * (sum(m * img) / sum(m))        (masked mean over the 16x16 patch)

Sharding: data-parallel over the flattened (batch * n_areas) axis across 8
NeuronCores; no cross-core communication.

Math notes:
  diff_round(x) = x - sin(2*pi*x)/(2*pi).  Work in "y-space" (y = 2*pi*x):
  f(y) = y - sin(y); harder_diff_round(x) = f(f(f(2*pi*x)))/(2*pi).
  The ScalarEngine Sin spline is valid only on [-pi, pi], so every sin(y) for
  y in [0, 2*pi] is computed as -sin(y - pi) via the activation's free affine
  (bias = -pi), turning all f-step subtracts into adds.
  differentiable_eq(a, B) with B = hdr(id) constant per (area, channel) is the
  affine  t = A*(2B-1) + (1-B)  of A = hdr(a); in y-space z = yA*S + U with
  S = 2B-1, U = 2*pi*(1-B), both precomputed on host (tiny).
  The masked mean is scale-invariant in m, so the pipeline carries
  m~ = (2*pi)^2 * m and only rescales in the final per-area multiply.
"""

import itertools

import numpy as np

import concourse.bacc as bacc
import concourse.mybir as mybir
import concourse.tile as tile
from concourse.bass_utils import run_bass_kernel_spmd

# ---------------------------------------------------------------- geometry
N_CORES = 8
B, N, DX, DY, C = 2, 8192, 16, 16, 4
PIX = DX * DY                      # 256 pixels per area
W_IN = PIX * C                     # 1024 mask values per area (channel-interleaved)
A_TOT = B * N                      # 16384 areas
A_CORE = A_TOT // N_CORES          # 2048 areas per core
P = 128                            # SBUF partitions

PI = float(np.pi)
TWO_PI = float(2.0 * np.pi)
EPS_GUARD = 2e-5                   # keeps sin args strictly inside [-pi, pi]
GA = 1.0 - EPS_GUARD
INV_4PI2 = float(1.0 / (4.0 * np.pi * np.pi))

F32 = mybir.dt.float32
BF16 = mybir.dt.bfloat16
SIN = mybir.ActivationFunctionType.Sin
COPY = mybir.ActivationFunctionType.Copy
MULT = mybir.AluOpType.mult
ADD = mybir.AluOpType.add
BYPASS = mybir.AluOpType.bypass
AX_X = mybir.AxisListType.X

# compute dtype for the bulk elementwise pipeline ("f32" or "bf16")
COMPUTE = "f32"
G = 2                              # areas per partition per mega-tile
BIG_BUFS = 4
MED_BUFS = 3
AND_BF16 = True                    # AND phase (w products onward) in bf16
CCE_STEPS = ()                     # f-step adds computed by DMA CCE accumulate


def build(nc, a_core=A_CORE, g=G, compute=COMPUTE):
    """Emit the Tile graph onto `nc` for one core's shard of `a_core` areas."""
    dt = F32 if compute == "f32" else BF16
    W = g * W_IN                   # mega-tile mask width (f32 elems per partition)
    Q = g * PIX                    # mega-tile single-channel width
    n_tiles = a_core // (P * g)
    assert n_tiles * P * g == a_core

    d_mask = nc.dram_tensor("mask", [a_core, W_IN], F32, kind="ExternalInput")
    d_alt = nc.dram_tensor("alt", [a_core, W_IN], F32, kind="ExternalInput")
    d_img = nc.dram_tensor("img", [a_core, PIX], F32, kind="ExternalInput")
    d_su = nc.dram_tensor("su", [a_core, 8], F32, kind="ExternalInput")
    d_out = nc.dram_tensor("out", [a_core, PIX], F32, kind="ExternalOutput")
    d_outa = nc.dram_tensor("outalt", [a_core, PIX], F32, kind="ExternalOutput")

    mask_v = d_mask.ap().rearrange("(t p g) f -> t p (g f)", p=P, g=g)
    alt_v = d_alt.ap().rearrange("(t p g) f -> t p (g f)", p=P, g=g)
    img_v = d_img.ap().rearrange("(t p g) f -> t p (g f)", p=P, g=g)
    su_v = d_su.ap().rearrange("(t p g) c -> p t g c", p=P, g=g)
    out_v = d_out.ap().rearrange("(t p g) f -> t p (g f)", p=P, g=g)
    outa_v = d_outa.ap().rearrange("(t p g) f -> t p (g f)", p=P, g=g)

    with tile.TileContext(nc) as tc:
        from contextlib import ExitStack

        with ExitStack() as ctx:
            const = ctx.enter_context(tc.tile_pool(name="const", bufs=1))
            big = ctx.enter_context(tc.tile_pool(name="big", bufs=BIG_BUFS))
            med = ctx.enter_context(tc.tile_pool(name="med", bufs=MED_BUFS))
            sm = ctx.enter_context(tc.tile_pool(name="sm", bufs=MED_BUFS))

            nb = const.tile([P, 1], F32, tag="nb")       # -pi*GA bias for sin
            nc.gpsimd.memset(nb[:], -PI * GA)
            su_sb = const.tile([P, n_tiles * g * 8], F32, tag="su")
            nc.sync.dma_start(
                su_sb[:].rearrange("p (t g c) -> p t g c", t=n_tiles, g=g), su_v
            )

            def f_step(y, tag, j, out_dt=None, cce=False):
                """y <- f(y) = y - sin(y), via s = -sin(y) then add."""
                s = big.tile([P, W], out_dt or dt, tag=f"sin{j}", bufs=2)
                nc.scalar.activation(s[:], y[:], SIN, scale=GA, bias=nb[:])
                if cce:
                    # accumulate in place on the DMA engines (CCE inline add);
                    # frees the VectorEngine at the cost of SBUF fabric traffic
                    nc.gpsimd.dma_start(y[:], s[:], accum_op=ADD)
                    return y
                y2 = big.tile([P, W], out_dt or dt, tag=f"{tag}{j}", bufs=3 if tag == "zz" else 2)
                nc.vector.tensor_tensor(y2[:], y[:], s[:], ADD)
                return y2

            def emit_pass(t, j, img_c):
                src_v, dst_v = ((mask_v, out_v), (alt_v, outa_v))[j]
                x = big.tile([P, W], F32, tag="x", bufs=3)
                nc.sync.dma_start(x[:], src_v[t])

                # ---- A phase: y3 = f^3(2*pi*x)  (hdr of mask, y-space)
                s0 = big.tile([P, W], dt, tag=f"sin{j}", bufs=2)
                nc.scalar.activation(s0[:], x[:], SIN, scale=TWO_PI * GA, bias=nb[:])
                y1 = big.tile([P, W], dt, tag=f"yy{j}", bufs=2)
                if compute == "f32":
                    nc.vector.scalar_tensor_tensor(
                        y1[:], x[:], TWO_PI, s0[:], MULT, ADD
                    )
                else:
                    y0 = big.tile([P, W], dt, tag=f"y0{j}")
                    nc.scalar.activation(y0[:], x[:], COPY, scale=TWO_PI)
                    nc.vector.tensor_tensor(y1[:], y0[:], s0[:], ADD)
                y2 = f_step(y1, "yy", j, cce="y2" in CCE_STEPS)
                y3 = f_step(y2, "yy", j)
                yield

                # ---- eq phase: z = y3*S + U per (area, channel),
                # de-interleaving to channel-major [c][g][pix] layout via
                # strided reads (strided reads are full-rate on the DVE)
                z = big.tile([P, W], dt, tag=f"zz{j}", bufs=3)
                y3v = y3[:].rearrange("p (g i c) -> p g c i", g=g, c=C)
                zv = z[:].rearrange("p (c g i) -> p c g i", c=C, g=g)
                for gg in range(g):
                    col = (t * g + gg) * 8
                    for c in range(C):
                        nc.vector.tensor_scalar(
                            zv[:, c, gg, :],
                            y3v[:, gg, c, :],
                            su_sb[:, col + c : col + c + 1],
                            su_sb[:, col + 4 + c : col + 4 + c + 1],
                            MULT,
                            ADD,
                        )
                # f^3 -> e (y-space eq), then w = f(e) = 2*pi*dr(eq)
                e1 = f_step(z, "zz", j)
                e2 = f_step(e1, "zz", j)
                e3 = f_step(e2, "zz", j, cce="e3" in CCE_STEPS)
                w = f_step(e3, "zz", j, out_dt=BF16 if AND_BF16 else None)
                yield

                # ---- AND phase (channel-major blocks are contiguous)
                adt = BF16 if AND_BF16 else dt
                ab = med.tile([P, 2 * Q], adt, tag="ab")
                nc.vector.tensor_tensor(ab[:, 0:Q], w[:, 0:Q], w[:, Q : 2 * Q], MULT)
                nc.vector.tensor_tensor(
                    ab[:, Q : 2 * Q], w[:, 2 * Q : 3 * Q], w[:, 3 * Q : 4 * Q], MULT
                )
                sab = med.tile([P, 2 * Q], adt, tag="sab")
                nc.scalar.activation(sab[:], ab[:], SIN, scale=GA / TWO_PI, bias=nb[:])
                fab = med.tile([P, 2 * Q], adt, tag="fab")
                nc.vector.scalar_tensor_tensor(
                    fab[:], ab[:], 1.0 / TWO_PI, sab[:], MULT, ADD
                )
                fa, fb = fab[:, 0:Q], fab[:, Q : 2 * Q]

                den = sm.tile([P, g], F32, tag="den")
                num = sm.tile([P, g], F32, tag="num")
                m = med.tile([P, Q], adt, tag="mm")
                mi = med.tile([P, Q], adt, tag="mi")
                for gg in range(g):
                    gs = slice(gg * PIX, (gg + 1) * PIX)
                    nc.vector.scalar_tensor_tensor(
                        m[:, gs], fa[:, gs], 0.0, fb[:, gs], BYPASS, MULT,
                        accum_out=den[:, gg : gg + 1],
                    )
                    nc.vector.scalar_tensor_tensor(
                        mi[:, gs], m[:, gs], 0.0, img_c[:, gs], BYPASS, MULT,
                        accum_out=num[:, gg : gg + 1],
                    )
                rd = sm.tile([P, g], F32, tag="rd")
                nc.vector.reciprocal(rd[:], den[:])
                q = sm.tile([P, g], F32, tag="qq")
                nc.vector.tensor_tensor(q[:], num[:], rd[:], MULT)

                o = med.tile([P, Q], F32, tag="oo")
                for gg in range(g):
                    nc.vector.tensor_scalar(
                        o[:, gg * PIX : (gg + 1) * PIX],
                        m[:, gg * PIX : (gg + 1) * PIX],
                        q[:, gg : gg + 1],
                        INV_4PI2,
                        MULT,
                        MULT,
                    )
                nc.sync.dma_start(dst_v[t], o[:])
                yield

            for t in range(n_tiles):
                img_sb = med.tile([P, Q], F32, tag="img")
                nc.sync.dma_start(img_sb[:], img_v[t])
                if AND_BF16 or compute != "f32":
                    img_c = med.tile([P, Q], BF16 if AND_BF16 else dt, tag="imgc")
                    nc.vector.tensor_copy(img_c[:], img_sb[:])
                else:
                    img_c = img_sb
                # interleave the two independent mask pipelines phase-by-phase
                for _ in itertools.zip_longest(
                    emit_pass(t, 0, img_c), emit_pass(t, 1, img_c)
                ):
                    pass

    return nc


# ------------------------------------------------------------- host helpers
def _hdr_np(x):
    def dr(v):
        return v - np.sin(2.0 * np.pi * v) / (2.0 * np.pi)

    return dr(dr(dr(x)))


_NC_CACHE = {}


def _get_compiled():
    key = (COMPUTE, G)
    if key not in _NC_CACHE:
        nc = bacc.Bacc(
            "TRN2", target_bir_lowering=False, debug=False, num_devices=N_CORES
        )
        build(nc, A_CORE, G, COMPUTE)
        nc.compile()
        _NC_CACHE[key] = nc
    return _NC_CACHE[key]


def _make_in_maps(resized_image, mask_combined, mask_combined_alt, initial_mask_id):
    mask = np.ascontiguousarray(
        np.asarray(mask_combined, dtype=np.float32).reshape(A_TOT, W_IN)
    )
    alt = np.ascontiguousarray(
        np.asarray(mask_combined_alt, dtype=np.float32).reshape(A_TOT, W_IN)
    )
    img = np.ascontiguousarray(
        np.asarray(resized_image, dtype=np.float32).reshape(A_TOT, PIX)
    )
    idf = np.asarray(initial_mask_id, dtype=np.float64).reshape(A_TOT, C)
    bh = _hdr_np(idf)
    su = np.concatenate(
        [2.0 * bh - 1.0, 2.0 * np.pi * (1.0 - bh)], axis=1
    ).astype(np.float32)

    in_maps = []
    for k in range(N_CORES):
        sl = slice(k * A_CORE, (k + 1) * A_CORE)
        in_maps.append(
            {"mask": mask[sl], "alt": alt[sl], "img": img[sl], "su": su[sl]}
        )
    return in_maps


def run(inputs, trace=False, trace_kwargs=None):
    """Run the kernel on all 8 cores; returns ((out, out_alt), exec_time_ns)."""
    nc = _get_compiled()
    in_maps = _make_in_maps(
        inputs["resized_image"],
        inputs["mask_combined"],
        inputs["mask_combined_alt"],
        inputs["initial_mask_id"],
    )
    res = run_bass_kernel_spmd(
        nc,
        in_maps,
        list(range(N_CORES)),
        trace=trace,
        **(trace_kwargs or {}),
    )
    out = np.empty((A_TOT, PIX), np.float32)
    outa = np.empty((A_TOT, PIX), np.float32)
    for k in range(N_CORES):
        sl = slice(k * A_CORE, (k + 1) * A_CORE)
        out[sl] = res.results[k]["out"]
        outa[sl] = res.results[k]["outalt"]
    shape = (B, N, DX, DY, 1)
    return (out.reshape(shape), outa.reshape(shape)), res.exec_time_ns


def kernel(**inputs):
    (out, outa), _ = run(inputs, trace=False)
    return out, outa
